# revision 8
# baseline (speedup 1.0000x reference)
"""Causal single-head attention (B=8, T=2048, C=1024, HS=64) on 8 trn2 cores.

Strategy: data-parallel over batch B — one batch element per NeuronCore.
Per core:
  1. Cast-load q/k/v [T,C] fp32 -> SBUF bf16 natural tiles (SWDGE cast DMA).
  2. Xbar DMA-transpose each [128,1024] tile -> [C-part, T-free] layout
     (projections contract over C, which must live on partitions).
  3. Projections on PE (bf16, fp32 PSUM accum): Q^T,K^T,V^T [64, T].
  4. V^T is PE-transposed back to V [T,64] and extended with a mask/ones
     column so the softmax denominator falls out of the A@V matmul.
  5. Causal attention in fp32r: scores^T[j,i] per (i-chunk, j-block),
     exp on ScalarE (scale=1/8 fused; no max subtraction -- scores are
     ~N(0,1), fp32 exp cannot overflow), diagonal-block causal mask via a
     multiplicative upper-triangular tile, then out_un^T accumulated in
     PSUM over j-blocks.
  6. PE-transpose out_un^T, divide by the denominator column, DMA out.
"""

import numpy as np

import concourse.bass as bass
import concourse.mybir as mybir
import concourse.tile as tile
from concourse.masks import make_identity, make_upper_triangular

B, T, C, HS = 8, 2048, 1024, 64
P = 128
NT = T // P  # 16 t-tiles
NCB = C // P  # 8 c-chunks
TI = 512  # i-chunk width
NIC = T // TI  # 4 i-chunks

F32 = mybir.dt.float32
F32R = mybir.dt.float32r
BF16 = mybir.dt.bfloat16
I32 = mybir.dt.int32


def split_excess_waits(nc):
    """walrus supports 1 sync-wait per instruction (2 on EventSemaphore);
    Tile's final drain can accumulate more. Hoist excess waits onto NoOp
    carriers inserted immediately before the overloaded instruction."""
    for blk in nc.m.functions[0].blocks:
        insts = blk.instructions
        i = 0
        while i < len(insts):
            inst = insts[i]
            si = inst.sync_info
            cap = 2 if isinstance(inst, mybir.InstEventSemaphore) else 1
            if si is not None and si.on_wait and len(si.on_wait) > cap:
                waits = list(si.on_wait)
                si.on_wait = waits[:cap]
                carriers = []
                for w in waits[cap:]:
                    n = mybir.InstNoOp(
                        name=nc.get_next_instruction_name(), ins=[], outs=[]
                    )
                    n.engine = inst.engine
                    n.sync_info = mybir.SyncInfo(on_wait=[w], on_update=[])
                    nc.register_instruction(n)
                    carriers.append(n)
                for j, n in enumerate(carriers):
                    insts.insert(i + j, n)
                i += len(carriers)
            i += 1


def attention_body(tc, q, k, v, mask, wq, wk, wv, out):
    """Emit one iteration of the attention kernel (per-core shapes)."""
    nc = tc.nc
    from contextlib import ExitStack

    with ExitStack() as ctx:
        singles = ctx.enter_context(tc.tile_pool(name="singles", bufs=1))
        nat_pool = ctx.enter_context(tc.tile_pool(name="nat", bufs=4))
        xt_pool = ctx.enter_context(tc.tile_pool(name="xt", bufs=1))
        proj_pool = ctx.enter_context(tc.tile_pool(name="proj", bufs=1))
        ps_proj = ctx.enter_context(tc.tile_pool(name="ps_proj", bufs=2, space="PSUM"))
        ps_sc = ctx.enter_context(tc.tile_pool(name="ps_sc", bufs=2, space="PSUM"))
        ps_out = ctx.enter_context(tc.tile_pool(name="ps_out", bufs=2, space="PSUM"))
        ps_tr = ctx.enter_context(tc.tile_pool(name="ps_tr", bufs=2, space="PSUM"))
        exp_pool = ctx.enter_context(tc.tile_pool(name="exp", bufs=3))
        misc_pool = ctx.enter_context(tc.tile_pool(name="misc", bufs=4))

        # --- constants ---
        ident = singles.tile([P, P], F32)
        make_identity(nc, ident[:])
        # umask[jj, ii] = 1 where ii >= jj else 0 (keep causal i >= j)
        umask = singles.tile([P, P], F32)
        make_upper_triangular(nc, umask[:], val=1.0, diag=True)

        # weights [C, HS] fp32 -> bf16 chunks [128, cb, HS]
        w_sb = []
        for name, w in (("wq", wq), ("wk", wk), ("wv", wv)):
            t_ = singles.tile([P, NCB, HS], BF16, tag=f"w_{name}")
            nc.gpsimd.dma_start(
                out=t_[:], in_=w.rearrange("(cb c) h -> c cb h", c=P)
            )
            w_sb.append(t_)

        # mask [T] int32 -> [128, NT] fp32
        mask_i = singles.tile([P, NT], I32)
        nc.gpsimd.dma_start(out=mask_i[:], in_=mask.rearrange("(tb p) -> p tb", p=P))
        mask_f = singles.tile([P, NT], F32)
        nc.vector.tensor_copy(out=mask_f[:], in_=mask_i[:])

        # --- load + transpose inputs ---
        # xT[c, tb, cb, t] = x[128*tb + t, 128*cb + c], bf16
        xts = []
        for name, x in (("q", q), ("k", k), ("v", v)):
            xt = xt_pool.tile([P, NT, NCB, P], BF16, tag=f"xt_{name}")
            for tb in range(NT):
                nat = nat_pool.tile([P, C], BF16, tag="nat")
                nc.gpsimd.dma_start(out=nat[:], in_=x[tb * P : (tb + 1) * P, :])
                nc.sync.dma_start_transpose(out=xt[:, tb], in_=nat[:])
            xts.append(xt)

        # --- projections: XT[h, t] = W.T @ x^T, accumulated over c-chunks ---
        projs = []
        for idx, name in enumerate(("q", "k", "v")):
            pt = proj_pool.tile([HS, T], F32R if name != "v" else F32, tag=f"p_{name}")
            for oc in range(NIC):
                pps = ps_proj.tile([HS, TI], F32, tag="ps_proj")
                for cb in range(NCB):
                    nc.tensor.matmul(
                        pps[:],
                        lhsT=w_sb[idx][:, cb, :],
                        rhs=xts[idx][:, 4 * oc : 4 * oc + 4, cb, :],
                        start=(cb == 0),
                        stop=(cb == NCB - 1),
                    )
                nc.vector.tensor_copy(out=pt[:, oc * TI : (oc + 1) * TI], in_=pps[:])
            projs.append(pt)
        qt, kt, vt = projs

        # --- V_ext[j, tb, 0:64] = V[j]*mask[j]; V_ext[j, tb, 64] = mask[j] ---
        vx = proj_pool.tile([P, NT, HS + 1], F32R, tag="vx")
        for tb in range(NT):
            vt_ps = ps_tr.tile([P, HS + 1], F32, tag="tr_ps")
            nc.tensor.transpose(
                vt_ps[:, 0:HS], vt[:, tb * P : (tb + 1) * P], ident[0:HS, 0:HS]
            )
            nc.vector.tensor_scalar_mul(
                out=vx[:, tb, 0:HS], in0=vt_ps[:, 0:HS], scalar1=mask_f[:, tb : tb + 1]
            )
            nc.vector.tensor_copy(out=vx[:, tb, HS : HS + 1], in_=mask_f[:, tb : tb + 1])

        qt_r = qt[:]
        kt_r = kt[:]
        vx_r = vx[:]

        # --- causal attention ---
        for ic in range(NIC):
            njb = 4 * ic + 4
            out_ps = ps_out.tile([HS + 1, TI], F32, tag="out_ps")
            for jb in range(njb):
                o = max(0, jb * P - ic * TI)
                w = TI - o
                sc_ps = ps_sc.tile([P, TI], F32, tag="sc_ps")
                nc.tensor.matmul(
                    sc_ps[:, :w],
                    lhsT=kt_r[:, jb * P : (jb + 1) * P],
                    rhs=qt_r[:, ic * TI + o : (ic + 1) * TI],
                    start=True,
                    stop=True,
                )
                ex = exp_pool.tile([P, TI], F32R, tag="ex")
                nc.scalar.activation(
                    out=ex[:, :w],
                    in_=sc_ps[:, :w],
                    func=mybir.ActivationFunctionType.Exp,
                    scale=float(HS) ** -0.5,
                )
                if jb >= 4 * ic:
                    # diagonal block: zero out j > i entries
                    nc.vector.tensor_mul(ex[:, 0:P], ex[:, 0:P], umask[:])
                nc.tensor.matmul(
                    out_ps[:, o:],
                    lhsT=vx_r[:, jb, :],
                    rhs=ex[:, :w],
                    start=(jb == 0),
                    stop=(jb == njb - 1),
                )
            # normalize + emit
            oun = misc_pool.tile([HS + 1, TI], F32, tag="oun")
            nc.vector.tensor_copy(out=oun[:], in_=out_ps[:])
            for tt in range(TI // P):
                ot_ps = ps_tr.tile([P, HS + 1], F32, tag="tr_ps")
                nc.tensor.transpose(
                    ot_ps[:], oun[:, tt * P : (tt + 1) * P], ident[0 : HS + 1, 0 : HS + 1]
                )
                rden = misc_pool.tile([P, 1], F32, tag="rden")
                nc.vector.reciprocal(out=rden[:], in_=ot_ps[:, HS : HS + 1])
                osb = misc_pool.tile([P, HS], F32, tag="osb")
                nc.vector.tensor_scalar_mul(out=osb[:], in0=ot_ps[:, 0:HS], scalar1=rden[:])
                row = ic * TI + tt * P
                nc.sync.dma_start(out=out[row : row + P, :], in_=osb[:])


def build_nc(n_iters: int = 1):
    nc = bass.Bass(trn_type="TRN2", num_devices=B)
    q = nc.declare_dram_parameter("q_vec", [T, C], F32, isOutput=False)
    k = nc.declare_dram_parameter("k_vec", [T, C], F32, isOutput=False)
    v = nc.declare_dram_parameter("v_vec", [T, C], F32, isOutput=False)
    mask = nc.declare_dram_parameter("mask", [T], I32, isOutput=False)
    wq = nc.declare_dram_parameter("Wq", [C, HS], F32, isOutput=False)
    wk = nc.declare_dram_parameter("Wk", [C, HS], F32, isOutput=False)
    wv = nc.declare_dram_parameter("Wv", [C, HS], F32, isOutput=False)
    out = nc.declare_dram_parameter("out", [T, HS], F32, isOutput=True)

    with tile.TileContext(nc) as tc:
        for _ in range(n_iters):
            attention_body(
                tc, q.ap(), k.ap(), v.ap(), mask.ap(),
                wq.ap(), wk.ap(), wv.ap(), out.ap(),
            )

    split_excess_waits(nc)
    return nc


# ---------------------------------------------------------------------------
# SPMD runner (compile once, execute via PJRT on the 8 axon cores)
# ---------------------------------------------------------------------------
class _Runner:
    def __init__(self, nc, n_cores=B):
        import jax
        from jax.sharding import Mesh, PartitionSpec
        from jax.experimental.shard_map import shard_map
        from concourse.bass2jax import (
            _bass_exec_p,
            install_neuronx_cc_hook,
            partition_id_tensor,
        )

        install_neuronx_cc_hook()
        self.jax = jax
        self.n_cores = n_cores
        partition_name = (
            nc.partition_id_tensor.name if nc.partition_id_tensor else None
        )

        in_names, out_names, out_avals, zero_outs = [], [], [], []
        for alloc in nc.m.functions[0].allocations:
            if not isinstance(alloc, mybir.MemoryLocationSet):
                continue
            name = alloc.memorylocations[0].name
            if alloc.kind == "ExternalInput":
                if name != partition_name:
                    in_names.append(name)
            elif alloc.kind == "ExternalOutput":
                out_names.append(name)
                shape = tuple(alloc.tensor_shape)
                dtype = mybir.dt.np(alloc.dtype)
                out_avals.append(jax.core.ShapedArray(shape, dtype))
                zero_outs.append(np.zeros(shape, dtype))
        self.in_names = list(in_names)
        self.out_names = out_names
        self.out_avals = out_avals
        self.zero_outs = zero_outs
        n_params = len(in_names)
        self.n_params = n_params

        all_in_names = list(in_names) + list(out_names)
        if partition_name is not None:
            all_in_names.append(partition_name)

        def _body(*args):
            operands = list(args)
            if partition_name is not None:
                operands.append(partition_id_tensor())
            outs = _bass_exec_p.bind(
                *operands,
                out_avals=tuple(out_avals),
                in_names=tuple(all_in_names),
                out_names=tuple(out_names),
                lowering_input_output_aliases=(),
                sim_require_finite=True,
                sim_require_nnan=True,
                nc=nc,
            )
            return tuple(outs)

        devices = jax.devices()[:n_cores]
        mesh = Mesh(np.asarray(devices), ("core",))
        n_outs = len(out_names)
        self.fn = jax.jit(
            shard_map(
                _body,
                mesh=mesh,
                in_specs=(PartitionSpec("core"),) * (n_params + n_outs),
                out_specs=(PartitionSpec("core"),) * n_outs,
                check_rep=False,
            ),
            keep_unused=True,
        )

    def prepare(self, in_maps):
        n = self.n_cores
        per_core = [[np.asarray(m[nm]) for nm in self.in_names] for m in in_maps]
        concat_in = [
            np.concatenate([per_core[c][i] for c in range(n)], axis=0)
            for i in range(self.n_params)
        ]
        concat_zeros = [
            np.zeros((n * z.shape[0], *z.shape[1:]), z.dtype) for z in self.zero_outs
        ]
        self.args = [self.jax.device_put(a) for a in concat_in + concat_zeros]
        return self

    def run(self):
        outs = self.fn(*self.args)
        self.jax.block_until_ready(outs)
        return outs

    def results(self, outs):
        n = self.n_cores
        return [
            {
                nm: np.asarray(outs[i]).reshape(n, *self.out_avals[i].shape)[c]
                for i, nm in enumerate(self.out_names)
            }
            for c in range(n)
        ]


_CACHED = {}


def _get_runner(n_iters: int = 1):
    if n_iters not in _CACHED:
        _CACHED[n_iters] = _Runner(build_nc(n_iters))
    return _CACHED[n_iters]


def kernel(q_vec, k_vec, v_vec, mask, Wq, Wk, Wv):
    q_vec = np.ascontiguousarray(np.asarray(q_vec, dtype=np.float32))
    k_vec = np.ascontiguousarray(np.asarray(k_vec, dtype=np.float32))
    v_vec = np.ascontiguousarray(np.asarray(v_vec, dtype=np.float32))
    mask = np.ascontiguousarray(np.asarray(mask, dtype=np.int32))
    Wq = np.ascontiguousarray(np.asarray(Wq, dtype=np.float32))
    Wk = np.ascontiguousarray(np.asarray(Wk, dtype=np.float32))
    Wv = np.ascontiguousarray(np.asarray(Wv, dtype=np.float32))

    r = _get_runner()
    in_maps = [
        {
            "q_vec": q_vec[b],
            "k_vec": k_vec[b],
            "v_vec": v_vec[b],
            "mask": mask[b],
            "Wq": Wq,
            "Wk": Wk,
            "Wv": Wv,
        }
        for b in range(B)
    ]
    r.prepare(in_maps)
    res = r.results(r.run())
    return np.stack([res[b]["out"] for b in range(B)], axis=0)
